# revision 49
# baseline (speedup 1.0000x reference)
"""Trainium2 Bass kernel for nn_BlockBlastValueNet1PmultikernelFlattenned.

Strategy
--------
The network is 8 tiny conv branches over an 8x8 board followed by small MLPs.
Because the board has only 64 pixels, every conv branch (pad const 1.0 +
valid conv + bias) is an affine map of the 64 board values.  The whole net
folds into:

    y  = x @ W1 + c1                     # [B, NF]  (NF = 2944 padded)
    h  = Lrelu( Lrelu(y) @ W2' + b2 )    # per-branch first FC, block diagonal
    g1 = Lrelu( h @ W3 + b3 )            # branch second FC fused with fc1
    g2 = Lrelu( g1 @ W4 + b4 )           # fc2 (augmented with a ones column)
    out = g2 @ W5                        # fc3 (bias folded via augmentation)

The LeakyReLU between the two big matmuls is decomposed as
    Lrelu(v) = 0.01*v + 0.99*relu(v)
so the "0.01*v" part collapses into a small 64->128 matmul (W12, emitted as
a row-duplicated K=128 chain opener) and only relu(y) is materialized, by a
one-op PSUM->SBUF evacuation split between the Scalar and Vector engines.

Data-parallel over 8 NeuronCores (batch 65536 -> 8192/core), feature-major
layout (features on SBUF partitions, samples streaming on the free dim),
processed in 1024-sample pairs.

PE schedule (all matmuls fp16, N=512 chunks):
  * step-1 (x -> y): K=64 M-tile pairs row-packed at tile_position
    (0,0)/(64,0), issued alternating A0,B0,A1,B1 so the two 64-row halves
    dual-stream through disjoint partition lanes (2 x 64 = 128 lanes/cycle).
  * step-2 (relu(y) -> h): W2 is block diagonal -- each y feature feeds only
    its branch's 16 h units.  The 8 branches are split into two sets of 4
    (h columns 0:64 and 64:128) and their K-tiles interleaved, so the
    accumulation chains become M=64 matmuls column-tiled at tile_position
    (0,0)/(0,64); an adjacent A/B pair of chain matmuls runs CONCURRENTLY
    in the two column halves of the PE array, halving step-2 wall time
    versus dense M=128 chains.  The W12 fold opens both chains as a single
    full-width K=128 matmul (one has_written clear per PSUM bank).
  * chain matmuls for a y-tile are emitted two slots after its evacuation
    so the in-order PE queue never parks behind a producer.
  * y evacuations: even global tiles on Vector, odd on Scalar.  The A-tile
    evacuations gate the next slot's PSUM buffer reuse, so they live on the
    engine with no tail-activation bursts; Scalar takes the rest plus the
    serial tail (h -> g1 -> g2 -> out), which is interleaved into the NEXT
    pair's slot stream one stage per slot starting at slot 2.
"""

import numpy as np

# ---------------------------------------------------------------- constants
SPECS = [(1, 1, 1, 0, 0), (2, 2, 6, 1, 1), (3, 3, 8, 1, 1), (4, 4, 8, 2, 2),
         (5, 5, 16, 2, 2), (8, 8, 32, 0, 0), (1, 8, 4, 0, 0), (8, 1, 4, 0, 0)]
BOARD = 8
B_TOTAL = 65536
N_CORES = 8
BC = B_TOTAL // N_CORES          # 8192 samples per core
PAIR_N = 1024                    # samples per pair-iteration
CHUNK = 512                      # matmul moving width (1 psum bank fp32)
N_PAIRS = BC // PAIR_N           # 8

_BR_N = []
for kh, kw, fs, ph, pw in SPECS:
    _BR_N.append((BOARD + 2 * ph - kh + 1) * (BOARD + 2 * pw - kw + 1) * fs)
_NF_TRUE = sum(_BR_N)            # 2830

# branch grouping for the column-split step-2: group A -> h cols 0:64,
# group B -> h cols 64:128.  Chosen to balance the two chains' tile counts
# (13 vs 10) while keeping 23 total tiles.
GROUP_A = [4, 1, 5, 6]           # 1024+486+32+32 = 1574 feats -> 13 tiles
GROUP_B = [0, 2, 3, 7]           # 64+512+648+32  = 1256 feats -> 10 tiles
_NA = sum(_BR_N[b] for b in GROUP_A)
_NB = sum(_BR_N[b] for b in GROUP_B)
NT_A = -(-_NA // 128)            # 13
NT_B = -(-_NB // 128)            # 10
KT = NT_A + NT_B                 # 23 K-tiles of y
NF = KT * 128                    # 2944
N_S1 = (KT + 1) // 2             # step-1 row-packed slots (12)

# global tile order interleaves the two groups so each step-1 slot produces
# one tile of each group and their chain matmuls pair up column-wise.
_A_G = [2 * i if i < min(NT_A, NT_B) else NT_B + i for i in range(NT_A)]
_B_G = [2 * j + 1 if j < min(NT_A, NT_B) else NT_A + j for j in range(NT_B)]
GRP = [0] * KT
for g in _B_G:
    GRP[g] = 1
GROUP_LAST_T = {0: _A_G[-1], 1: _B_G[-1]}
# h-unit layout: 16 units per branch, group A first
HPOS = {}
for _i, _b in enumerate(GROUP_A + GROUP_B):
    HPOS[_b] = 16 * _i

LRELU_NEG = 0.01
_AVG = LRELU_NEG                 # direct-path coefficient (0.01)
_DIF = 1.0 - LRELU_NEG           # relu-path coefficient (0.99)

# evac engine split: True -> Vector engine, False -> Scalar engine.
DVE_TILE = [(k % 2 == 0) for k in range(KT)]

CHAIN_LAG = 2                    # slots between y evac and its chain matmuls


# ---------------------------------------------------------------- host fold
def _fold_params(p):
    """Fold conv branches + MLPs into the dense pipeline weights (float64)."""
    n_of = _BR_N
    W1_of, c1_of = {}, {}
    for i, (kh, kw, fs, ph, pw) in enumerate(SPECS):
        Ho = BOARD + 2 * ph - kh + 1
        Wo = BOARD + 2 * pw - kw + 1
        cw = np.asarray(p[f"b{i}_cw"], np.float64)
        cb = np.asarray(p[f"b{i}_cb"], np.float64)
        W1 = np.zeros((64, n_of[i]))
        c1 = np.zeros((n_of[i],))
        for f in range(fs):
            for oh in range(Ho):
                for ow in range(Wo):
                    oi = (f * Ho + oh) * Wo + ow
                    c1[oi] += cb[f]
                    for u in range(kh):
                        for v in range(kw):
                            r, c = oh + u - ph, ow + v - pw
                            w = cw[f, 0, u, v]
                            if 0 <= r < 8 and 0 <= c < 8:
                                W1[r * 8 + c, oi] += w
                            else:
                                c1[oi] += w        # pad value is 1.0
        W1_of[i] = W1
        c1_of[i] = c1

    # scatter each group's packed feature run into the interleaved global
    # K-layout: feature j of a group run lives in the group's (j//128)-th
    # tile at global tile index _A_G/_B_G, offset j%128.
    W1p = np.zeros((64, NF))
    c1p = np.zeros((NF,))
    W2p = np.zeros((NF, 128))
    b2p = np.zeros((128,))
    for branches, gmap in ((GROUP_A, _A_G), (GROUP_B, _B_G)):
        off = 0
        for b in branches:
            n = n_of[b]
            j = np.arange(off, off + n)
            idx = np.array([gmap[v] for v in j // 128]) * 128 + j % 128
            W1p[:, idx] = W1_of[b]
            c1p[idx] = c1_of[b]
            hp = HPOS[b]
            W2p[idx, hp:hp + 16] = np.asarray(p[f"b{b}_w1"], np.float64).T
            b2p[hp:hp + 16] = np.asarray(p[f"b{b}_b1"], np.float64)
            off += n

    Wb = np.zeros((128, 64))
    bb = np.zeros((64,))
    for b in range(8):
        hp = HPOS[b]
        Wb[hp:hp + 16, 8 * b:8 * b + 8] = np.asarray(p[f"b{b}_w2"], np.float64).T
        bb[8 * b:8 * b + 8] = np.asarray(p[f"b{b}_b2"], np.float64)
    fc_w1 = np.asarray(p["fc_w1"], np.float64)
    fc_b1 = np.asarray(p["fc_b1"], np.float64)
    W3 = Wb @ fc_w1.T
    b3 = bb @ fc_w1.T + fc_b1
    fc_w2 = np.asarray(p["fc_w2"], np.float64)
    fc_b2 = np.asarray(p["fc_b2"], np.float64)
    fc_w3 = np.asarray(p["fc_w3"], np.float64)
    fc_b3 = np.asarray(p["fc_b3"], np.float64)
    W4 = np.zeros((64, 17)); W4[:, :16] = fc_w2.T
    b4 = np.zeros((17,)); b4[:16] = fc_b2; b4[16] = 1.0
    W5 = np.zeros((17,)); W5[:16] = fc_w3[0]; W5[16] = fc_b3[0]

    W5 = W5.reshape(17, 1)

    # relu decomposition folds: Lrelu(y) = 0.01*y + 0.99*relu(y)
    W2s = _DIF * W2p                               # weights for relu(y) path
    W12 = _AVG * (W1p @ W2p)                       # direct x -> h path
    b2f = _AVG * (c1p @ W2p) + b2p                 # bias folded into h evac

    f32 = np.float32
    f16 = np.float16
    dev = {}
    # step-1 weights packed for row-pairing: slot s holds M-tiles 2s | 2s+1
    w1 = np.zeros((128, N_S1, 128), f16)
    for s in range(N_S1):
        w1[0:64, s, :] = W1p[:, 128 * (2 * s):128 * (2 * s + 1)]
        if 2 * s + 1 < KT:
            w1[64:128, s, :] = W1p[:, 128 * (2 * s + 1):128 * (2 * s + 2)]
    dev["w1"] = w1
    c1t = np.zeros((128, KT), f32)
    for t in range(KT):
        c1t[:, t] = c1p[128 * t:128 * (t + 1)]
    dev["c1t"] = c1t
    # per-tile chain weights: only the tile's group's 64 h-columns (the
    # rest are zero by the block-diagonal structure)
    w2 = np.zeros((128, KT, 64), f16)
    for t in range(KT):
        gofs = 64 * GRP[t]
        blk = W2s[128 * t:128 * (t + 1), :]
        assert np.all(blk[:, 64 - gofs:128 - gofs] == 0.0)
        w2[:, t, :] = blk[:, gofs:gofs + 64]
    dev["w2"] = w2
    # K=128 chain opener: xx rows are duplicated, so half weights twice
    dev["w12"] = (np.vstack([W12, W12]) * 0.5).astype(f16)
    dev["b2f"] = b2f.reshape(128, 1).astype(f32)
    dev["w3"] = W3.astype(f16)
    dev["b3"] = b3.reshape(64, 1).astype(f32)
    dev["w4"] = W4.astype(f16)
    dev["b4"] = b4.reshape(17, 1).astype(f32)
    dev["w5"] = W5.astype(f16)
    return dev


# ---------------------------------------------------------------- device IR
def _build_nc(n_pairs=N_PAIRS):
    import concourse.mybir as mybir
    import concourse.tile as tile
    from concourse import bacc
    from contextlib import ExitStack

    dt = mybir.dt
    AF = mybir.ActivationFunctionType
    ALU = mybir.AluOpType
    f32 = dt.float32
    f16 = dt.float16
    bc = n_pairs * PAIR_N

    nc = bacc.Bacc("TRN2", target_bir_lowering=False, debug=False,
                   num_devices=N_CORES)

    xx_d = nc.dram_tensor("xx", [128, bc], f16, kind="ExternalInput")
    w1_d = nc.dram_tensor("w1", [128, N_S1, 128], f16, kind="ExternalInput")
    c1t_d = nc.dram_tensor("c1t", [128, KT], f32, kind="ExternalInput")
    w2_d = nc.dram_tensor("w2", [128, KT, 64], f16, kind="ExternalInput")
    w12_d = nc.dram_tensor("w12", [128, 128], f16, kind="ExternalInput")
    b2f_d = nc.dram_tensor("b2f", [128, 1], f32, kind="ExternalInput")
    w3_d = nc.dram_tensor("w3", [128, 64], f16, kind="ExternalInput")
    b3_d = nc.dram_tensor("b3", [64, 1], f32, kind="ExternalInput")
    w4_d = nc.dram_tensor("w4", [64, 17], f16, kind="ExternalInput")
    b4_d = nc.dram_tensor("b4", [17, 1], f32, kind="ExternalInput")
    w5_d = nc.dram_tensor("w5", [17, 1], f16, kind="ExternalInput")
    o_d = nc.dram_tensor("o", [1, bc], f32, kind="ExternalOutput")

    with tile.TileContext(nc) as tc, ExitStack() as ctx:
        wpool = ctx.enter_context(tc.tile_pool(name="wpool", bufs=1))
        xpool = ctx.enter_context(tc.tile_pool(name="xpool", bufs=3))
        ypool = ctx.enter_context(tc.tile_pool(name="ypool", bufs=10))
        spool = ctx.enter_context(tc.tile_pool(name="spool", bufs=2))
        ps1p = ctx.enter_context(tc.tile_pool(name="ps1p", bufs=3, space="PSUM"))
        ps2p = ctx.enter_context(tc.tile_pool(name="ps2p", bufs=1, space="PSUM"))

        # pair-0 input first so compute can start while the rest streams in.
        # w1 is split so the first slots' weights land quickly; w2 is split
        # so pair-0's first chain tiles don't wait for the whole load.
        xx_first = xpool.tile([128, PAIR_N], f16, tag="xx", name="xx_first")
        nc.sync.dma_start(xx_first[:, 0:CHUNK], xx_d[:, 0:CHUNK])
        nc.sync.dma_start(xx_first[:, CHUNK:PAIR_N], xx_d[:, CHUNK:PAIR_N])
        w1_t = wpool.tile([128, N_S1, 128], f16)
        nc.gpsimd.dma_start(w1_t[:, 0:1, :], w1_d[:, 0:1, :])
        nc.gpsimd.dma_start(w1_t[:, 1:3, :], w1_d[:, 1:3, :])
        c1t_t = wpool.tile([128, KT], f32)
        nc.gpsimd.dma_start(c1t_t[:, 0:6], c1t_d[:, 0:6])
        w12_t = wpool.tile([128, 128], f16)
        nc.gpsimd.dma_start(w12_t[:], w12_d[:])
        nc.gpsimd.dma_start(w1_t[:, 3:N_S1, :], w1_d[:, 3:N_S1, :])
        nc.gpsimd.dma_start(c1t_t[:, 6:KT], c1t_d[:, 6:KT])
        w2_t = wpool.tile([128, KT, 64], f16)
        nc.gpsimd.dma_start(w2_t[:, 0:8, :], w2_d[:, 0:8, :])
        nc.gpsimd.dma_start(w2_t[:, 8:KT, :], w2_d[:, 8:KT, :])
        b2f_t = wpool.tile([128, 1], f32)
        nc.gpsimd.dma_start(b2f_t[:], b2f_d[:])
        w3_t = wpool.tile([128, 64], f16)
        nc.gpsimd.dma_start(w3_t[:], w3_d[:])
        b3_t = wpool.tile([64, 1], f32)
        nc.gpsimd.dma_start(b3_t[:], b3_d[:])
        w4_t = wpool.tile([64, 17], f16)
        nc.gpsimd.dma_start(w4_t[:], w4_d[:])
        b4_t = wpool.tile([17, 1], f32)
        nc.gpsimd.dma_start(b4_t[:], b4_d[:])
        w5_t = wpool.tile([17, 1], f16)
        nc.gpsimd.dma_start(w5_t[:], w5_d[:])

        def make_tail_stages(p, h_t, final=False):
            """The per-pair serial tail (g1 -> g2 -> out) as stages that get
            interleaved into the NEXT pair's slot stream, so the ACT hops
            overlap matmuls instead of stalling the in-order PE queue.
            For the final pair (nothing left to interleave into) the
            activations are emitted per 512-chunk so chunk-1's activation
            overlaps chunk-0's next matmul instead of serializing."""
            st = {}
            acts = 2 if final else 1

            def ch(i):
                n = PAIR_N // acts
                return slice(i * n, (i + 1) * n)

            def s1():
                st["g1ps"] = ps1p.tile([64, PAIR_N], f32, tag="ps1",
                                       name=f"g1ps_{p}")
                for c in range(2):
                    sl = slice(c * CHUNK, (c + 1) * CHUNK)
                    nc.tensor.matmul(st["g1ps"][:, sl], w3_t[:],
                                     h_t[:, sl], start=True, stop=True)

            def s2():
                st["g1"] = spool.tile([64, PAIR_N], f16, tag="g1",
                                      name=f"g1_{p}")
                for i in range(acts):
                    nc.scalar.activation(st["g1"][:, ch(i)],
                                         st["g1ps"][:, ch(i)], AF.Lrelu,
                                         bias=b3_t[:, 0:1], alpha=LRELU_NEG)

            def s3():
                st["g2ps"] = ps1p.tile([17, PAIR_N], f32, tag="ps1",
                                       name=f"g2ps_{p}")
                for c in range(2):
                    sl = slice(c * CHUNK, (c + 1) * CHUNK)
                    nc.tensor.matmul(st["g2ps"][:, sl], w4_t[:],
                                     st["g1"][:, sl], start=True, stop=True)

            def s4():
                st["g2"] = spool.tile([17, PAIR_N], f16, tag="g2",
                                      name=f"g2_{p}")
                for i in range(acts):
                    nc.scalar.activation(st["g2"][:, ch(i)],
                                         st["g2ps"][:, ch(i)], AF.Lrelu,
                                         bias=b4_t[:, 0:1], alpha=LRELU_NEG)

            def s5():
                st["ops"] = ps1p.tile([1, PAIR_N], f32, tag="ps1",
                                      name=f"ops_{p}")
                for c in range(2):
                    sl = slice(c * CHUNK, (c + 1) * CHUNK)
                    nc.tensor.matmul(st["ops"][:, sl], w5_t[:],
                                     st["g2"][:, sl], start=True, stop=True)

            def s6():
                # single-partition copy is ~1.1us either way; Scalar has the
                # headroom (Vector owns the buffer-critical A evacuations)
                o_t = spool.tile([1, PAIR_N], f32, tag="o", name=f"o_{p}")
                for i in range(acts):
                    nc.scalar.activation(o_t[:, ch(i)], st["ops"][:, ch(i)],
                                         AF.Copy)
                nc.sync.dma_start(o_d[:, p * PAIR_N:(p + 1) * PAIR_N], o_t[:])

            return [s1, s2, s3, s4, s5, s6]

        tail_stages = []

        for p in range(n_pairs):
            if p == 0:
                xx_t = xx_first
            else:
                xx_t = xpool.tile([128, PAIR_N], f16, tag="xx", name=f"xx_{p}")
                nc.sync.dma_start(xx_t[:],
                                  xx_d[:, p * PAIR_N:(p + 1) * PAIR_N])

            # step-2 accumulator: single [128, 1024] PSUM tile; each 512
            # chunk holds two concurrent M=64 accumulation chains (group A
            # in partitions 0:64, group B in 64:128).
            ps2 = ps2p.tile([128, PAIR_N], f32, tag="ps2", name=f"ps2_{p}")

            def _emit_chains(tiles_done):
                if len(tiles_done) == 2 and GRP[tiles_done[0]] != GRP[tiles_done[1]]:
                    # A/B pair: order so adjacent matmuls sit in opposite
                    # column groups and dual-stream
                    t0, t1 = tiles_done
                    order = [(t0, 0), (t1, 0), (t1, 1), (t0, 1)]
                else:
                    order = [(t, c) for t in tiles_done for c in range(2)]
                for t, c in order:
                    g = GRP[t]
                    sl = slice(c * CHUNK, (c + 1) * CHUNK)
                    nc.tensor.matmul(
                        ps2[64 * g:64 * (g + 1), sl], w2_t[:, t, :],
                        ytiles[t][:, sl], start=False,
                        stop=(t == GROUP_LAST_T[g]),
                        tile_position=(0, 64 * g), skip_group_check=True)

            ytiles = [None] * KT
            pending = []
            for s in range(N_S1):
                # tail stages start at slot 2: the first stage's matmuls
                # depend on the h activation queued at the pair boundary,
                # and popping it earlier parks the in-order PE queue.
                if s >= 2 and tail_stages:
                    tail_stages.pop(0)()
                tA, tB = 2 * s, 2 * s + 1
                # Alternate which half leads per slot: odd slots emit the
                # B-phase first.  This lengthens every PSUM buffer-reuse
                # link in the 3-buffer rotation to ~1.5-2 slots with
                # matched dispatch phases (a fixed A-then-B order leaves
                # psB(s) <- psA(s-1) with almost no margin, and the
                # scheduler then phase-splits the slot, serializing the
                # dual-streamed pairs).  Within a slot the matmuls still
                # alternate row halves, so pairing is unchanged.
                halves = [(tA, 0), (tB, 64)] if tB < KT else [(tA, 0)]
                if s % 2 == 1 and tB < KT:
                    halves = halves[::-1]
                ps_of = {}
                for t, rofs in halves:
                    ps_of[t] = ps1p.tile([128, PAIR_N], f32, tag="ps1",
                                         name=f"ps_{p}_{t}")
                for c in range(2):
                    sl = slice(c * CHUNK, (c + 1) * CHUNK)
                    for t, rofs in halves:
                        nc.tensor.matmul(
                            ps_of[t][:, sl], w1_t[rofs:rofs + 64, s, :],
                            xx_t[rofs:rofs + 64, sl],
                            start=True, stop=True, tile_position=(rofs, 0))
                done = []
                for t, ps in [(t, ps_of[t]) for t, _ in halves]:
                    y_t = ypool.tile([128, PAIR_N], f16, tag="y",
                                     name=f"y_{p}_{t}")
                    if DVE_TILE[t]:
                        nc.vector.tensor_scalar(
                            y_t[:], ps[:], c1t_t[:, t:t + 1], 0.0,
                            ALU.add, ALU.max)
                    else:
                        nc.scalar.activation(
                            y_t[:], ps[:], AF.Relu, bias=c1t_t[:, t:t + 1])
                    ytiles[t] = y_t
                    done.append(t)
                pending.append(done)
                if s == CHAIN_LAG:
                    # full-width K=128 chain opener (row-duplicated W12):
                    # needs only xx, and its M=128 write sets has_written
                    # across both column groups' partition ranges.  Emitted
                    # at the first chain slot (still before the w2 chains
                    # below) so it never waits on the previous pair's h
                    # activation freeing the single-buffered ps2.
                    for c in range(2):
                        sl = slice(c * CHUNK, (c + 1) * CHUNK)
                        nc.tensor.matmul(
                            ps2[:, sl], w12_t[:], xx_t[:, sl],
                            start=True, stop=False, skip_group_check=True)
                if len(pending) > CHAIN_LAG:
                    _emit_chains(pending.pop(0))
            for done in pending:
                _emit_chains(done)

            # any remaining previous-pair tail stages
            for st in tail_stages:
                st()
            # h evacuation immediately at pair end: frees ps2 for the next
            # pair's chain opener (ps2 is single-buffered)
            h_t = spool.tile([128, PAIR_N], f16, tag="h", name=f"h_{p}")
            nc.scalar.activation(h_t[:], ps2[:], AF.Lrelu,
                                 bias=b2f_t[:, 0:1], alpha=LRELU_NEG)
            tail_stages = make_tail_stages(p, h_t, final=(p == n_pairs - 1))

        for st in tail_stages:
            st()

    nc.compile()
    return nc


# ---------------------------------------------------------------- execution
_NC_CACHE = {}
LAST_RESULT = None


def _prep_inputs(inputs):
    board = np.ascontiguousarray(np.asarray(inputs["board"], np.float32))
    x = board.reshape(B_TOTAL, 64)
    dev = _fold_params(inputs)
    in_maps = []
    for c in range(N_CORES):
        xc = np.ascontiguousarray(x[c * BC:(c + 1) * BC].T)      # [64, BC]
        m = dict(dev)
        m["xx"] = np.ascontiguousarray(
            np.vstack([xc, xc]).astype(np.float16))              # [128, BC]
        in_maps.append(m)
    return in_maps


def kernel(**inputs):
    global LAST_RESULT
    from concourse.bass_utils import run_bass_kernel_spmd

    if "nc" not in _NC_CACHE:
        _NC_CACHE["nc"] = _build_nc()
    nc = _NC_CACHE["nc"]

    in_maps = _prep_inputs(inputs)
    res = run_bass_kernel_spmd(nc, in_maps, core_ids=list(range(N_CORES)))
    LAST_RESULT = res
    out = np.concatenate([r["o"].reshape(-1) for r in res.results])
    return out.reshape(B_TOTAL, 1).astype(np.float32)


# revision 50
# speedup vs baseline: 1.0543x; 1.0543x over previous
"""Trainium2 Bass kernel for nn_BlockBlastValueNet1PmultikernelFlattenned.

Strategy
--------
The network is 8 tiny conv branches over an 8x8 board followed by small MLPs.
Because the board has only 64 pixels, every conv branch (pad const 1.0 +
valid conv + bias) is an affine map of the 64 board values.  The whole net
folds into:

    y  = x @ W1 + c1                     # [B, NF]  (NF = 2944 padded)
    h  = Lrelu( Lrelu(y) @ W2' + b2 )    # per-branch first FC, block diagonal
    g1 = Lrelu( h @ W3 + b3 )            # branch second FC fused with fc1
    g2 = Lrelu( g1 @ W4 + b4 )           # fc2 (augmented with a ones column)
    out = g2 @ W5                        # fc3 (bias folded via augmentation)

The LeakyReLU between the two big matmuls is decomposed as
    Lrelu(v) = 0.01*v + 0.99*relu(v)
so the "0.01*v" part collapses into a small 64->128 matmul (W12, emitted as
a row-duplicated K=128 chain opener) and only relu(y) is materialized, by a
one-op PSUM->SBUF evacuation split between the Scalar and Vector engines.

Data-parallel over 8 NeuronCores (batch 65536 -> 8192/core), feature-major
layout (features on SBUF partitions, samples streaming on the free dim),
processed in 1024-sample pairs.

PE schedule (all matmuls fp16, N=512 chunks):
  * step-1 (x -> y): K=64 M-tile pairs row-packed at tile_position
    (0,0)/(64,0), issued alternating A0,B0,A1,B1 so the two 64-row halves
    dual-stream through disjoint partition lanes (2 x 64 = 128 lanes/cycle).
  * step-2 (relu(y) -> h): W2 is block diagonal -- each y feature feeds only
    its branch's 16 h units.  The 8 branches are split into two sets of 4
    (h columns 0:64 and 64:128) and their K-tiles interleaved, so the
    accumulation chains become M=64 matmuls column-tiled at tile_position
    (0,0)/(0,64); an adjacent A/B pair of chain matmuls runs CONCURRENTLY
    in the two column halves of the PE array, halving step-2 wall time
    versus dense M=128 chains.  The W12 fold opens both chains as a single
    full-width K=128 matmul (one has_written clear per PSUM bank).
  * chain matmuls for a y-tile are emitted two slots after its evacuation
    so the in-order PE queue never parks behind a producer.
  * y evacuations: even global tiles on Vector, odd on Scalar.  The A-tile
    evacuations gate the next slot's PSUM buffer reuse, so they live on the
    engine with no tail-activation bursts; Scalar takes the rest plus the
    serial tail (h -> g1 -> g2 -> out), which is interleaved into the NEXT
    pair's slot stream one stage per slot starting at slot 2.
"""

import numpy as np

# ---------------------------------------------------------------- constants
SPECS = [(1, 1, 1, 0, 0), (2, 2, 6, 1, 1), (3, 3, 8, 1, 1), (4, 4, 8, 2, 2),
         (5, 5, 16, 2, 2), (8, 8, 32, 0, 0), (1, 8, 4, 0, 0), (8, 1, 4, 0, 0)]
BOARD = 8
B_TOTAL = 65536
N_CORES = 8
BC = B_TOTAL // N_CORES          # 8192 samples per core
PAIR_N = 1024                    # samples per pair-iteration
CHUNK = 512                      # matmul moving width (1 psum bank fp32)
N_PAIRS = BC // PAIR_N           # 8

_BR_N = []
for kh, kw, fs, ph, pw in SPECS:
    _BR_N.append((BOARD + 2 * ph - kh + 1) * (BOARD + 2 * pw - kw + 1) * fs)
_NF_TRUE = sum(_BR_N)            # 2830

# branch grouping for the column-split step-2: group A -> h cols 0:64,
# group B -> h cols 64:128.  Chosen to balance the two chains' tile counts
# (13 vs 10) while keeping 23 total tiles.
GROUP_A = [4, 1, 5, 6]           # 1024+486+32+32 = 1574 feats -> 13 tiles
GROUP_B = [0, 2, 3, 7]           # 64+512+648+32  = 1256 feats -> 10 tiles
_NA = sum(_BR_N[b] for b in GROUP_A)
_NB = sum(_BR_N[b] for b in GROUP_B)
NT_A = -(-_NA // 128)            # 13
NT_B = -(-_NB // 128)            # 10
KT = NT_A + NT_B                 # 23 K-tiles of y
NF = KT * 128                    # 2944
N_S1 = (KT + 1) // 2             # step-1 row-packed slots (12)

# global tile order interleaves the two groups so each step-1 slot produces
# one tile of each group and their chain matmuls pair up column-wise.
_A_G = [2 * i if i < min(NT_A, NT_B) else NT_B + i for i in range(NT_A)]
_B_G = [2 * j + 1 if j < min(NT_A, NT_B) else NT_A + j for j in range(NT_B)]
GRP = [0] * KT
for g in _B_G:
    GRP[g] = 1
GROUP_LAST_T = {0: _A_G[-1], 1: _B_G[-1]}
# h-unit layout: 16 units per branch, group A first
HPOS = {}
for _i, _b in enumerate(GROUP_A + GROUP_B):
    HPOS[_b] = 16 * _i

LRELU_NEG = 0.01
_AVG = LRELU_NEG                 # direct-path coefficient (0.01)
_DIF = 1.0 - LRELU_NEG           # relu-path coefficient (0.99)

# evac engine split: True -> Vector engine, False -> Scalar engine.
DVE_TILE = [(k % 2 == 0) for k in range(KT)]

CHAIN_LAG = 2                    # slots between y evac and its chain matmuls


# ---------------------------------------------------------------- host fold
def _fold_params(p):
    """Fold conv branches + MLPs into the dense pipeline weights (float64)."""
    n_of = _BR_N
    W1_of, c1_of = {}, {}
    for i, (kh, kw, fs, ph, pw) in enumerate(SPECS):
        Ho = BOARD + 2 * ph - kh + 1
        Wo = BOARD + 2 * pw - kw + 1
        cw = np.asarray(p[f"b{i}_cw"], np.float64)
        cb = np.asarray(p[f"b{i}_cb"], np.float64)
        W1 = np.zeros((64, n_of[i]))
        c1 = np.zeros((n_of[i],))
        for f in range(fs):
            for oh in range(Ho):
                for ow in range(Wo):
                    oi = (f * Ho + oh) * Wo + ow
                    c1[oi] += cb[f]
                    for u in range(kh):
                        for v in range(kw):
                            r, c = oh + u - ph, ow + v - pw
                            w = cw[f, 0, u, v]
                            if 0 <= r < 8 and 0 <= c < 8:
                                W1[r * 8 + c, oi] += w
                            else:
                                c1[oi] += w        # pad value is 1.0
        W1_of[i] = W1
        c1_of[i] = c1

    # scatter each group's packed feature run into the interleaved global
    # K-layout: feature j of a group run lives in the group's (j//128)-th
    # tile at global tile index _A_G/_B_G, offset j%128.
    W1p = np.zeros((64, NF))
    c1p = np.zeros((NF,))
    W2p = np.zeros((NF, 128))
    b2p = np.zeros((128,))
    for branches, gmap in ((GROUP_A, _A_G), (GROUP_B, _B_G)):
        off = 0
        for b in branches:
            n = n_of[b]
            j = np.arange(off, off + n)
            idx = np.array([gmap[v] for v in j // 128]) * 128 + j % 128
            W1p[:, idx] = W1_of[b]
            c1p[idx] = c1_of[b]
            hp = HPOS[b]
            W2p[idx, hp:hp + 16] = np.asarray(p[f"b{b}_w1"], np.float64).T
            b2p[hp:hp + 16] = np.asarray(p[f"b{b}_b1"], np.float64)
            off += n

    Wb = np.zeros((128, 64))
    bb = np.zeros((64,))
    for b in range(8):
        hp = HPOS[b]
        Wb[hp:hp + 16, 8 * b:8 * b + 8] = np.asarray(p[f"b{b}_w2"], np.float64).T
        bb[8 * b:8 * b + 8] = np.asarray(p[f"b{b}_b2"], np.float64)
    fc_w1 = np.asarray(p["fc_w1"], np.float64)
    fc_b1 = np.asarray(p["fc_b1"], np.float64)
    W3 = Wb @ fc_w1.T
    b3 = bb @ fc_w1.T + fc_b1
    fc_w2 = np.asarray(p["fc_w2"], np.float64)
    fc_b2 = np.asarray(p["fc_b2"], np.float64)
    fc_w3 = np.asarray(p["fc_w3"], np.float64)
    fc_b3 = np.asarray(p["fc_b3"], np.float64)
    W4 = np.zeros((64, 17)); W4[:, :16] = fc_w2.T
    b4 = np.zeros((17,)); b4[:16] = fc_b2; b4[16] = 1.0
    W5 = np.zeros((17,)); W5[:16] = fc_w3[0]; W5[16] = fc_b3[0]

    W5 = W5.reshape(17, 1)

    # relu decomposition folds: Lrelu(y) = 0.01*y + 0.99*relu(y)
    W2s = _DIF * W2p                               # weights for relu(y) path
    W12 = _AVG * (W1p @ W2p)                       # direct x -> h path
    b2f = _AVG * (c1p @ W2p) + b2p                 # bias folded into h evac

    f32 = np.float32
    f16 = np.float16
    dev = {}
    # step-1 weights packed for row-pairing: slot s holds M-tiles 2s | 2s+1
    w1 = np.zeros((128, N_S1, 128), f16)
    for s in range(N_S1):
        w1[0:64, s, :] = W1p[:, 128 * (2 * s):128 * (2 * s + 1)]
        if 2 * s + 1 < KT:
            w1[64:128, s, :] = W1p[:, 128 * (2 * s + 1):128 * (2 * s + 2)]
    dev["w1"] = w1
    c1t = np.zeros((128, KT), f32)
    for t in range(KT):
        c1t[:, t] = c1p[128 * t:128 * (t + 1)]
    dev["c1t"] = c1t
    # per-tile chain weights: only the tile's group's 64 h-columns (the
    # rest are zero by the block-diagonal structure)
    w2 = np.zeros((128, KT, 64), f16)
    for t in range(KT):
        gofs = 64 * GRP[t]
        blk = W2s[128 * t:128 * (t + 1), :]
        assert np.all(blk[:, 64 - gofs:128 - gofs] == 0.0)
        w2[:, t, :] = blk[:, gofs:gofs + 64]
    dev["w2"] = w2
    # K=128 chain opener: xx rows are duplicated, so half weights twice
    dev["w12"] = (np.vstack([W12, W12]) * 0.5).astype(f16)
    dev["b2f"] = b2f.reshape(128, 1).astype(f32)
    dev["w3"] = W3.astype(f16)
    dev["b3"] = b3.reshape(64, 1).astype(f32)
    dev["w4"] = W4.astype(f16)
    dev["b4"] = b4.reshape(17, 1).astype(f32)
    dev["w5"] = W5.astype(f16)
    return dev


# ---------------------------------------------------------------- device IR
def _build_nc(n_pairs=N_PAIRS):
    import concourse.mybir as mybir
    import concourse.tile as tile
    from concourse import bacc
    from contextlib import ExitStack

    dt = mybir.dt
    AF = mybir.ActivationFunctionType
    ALU = mybir.AluOpType
    f32 = dt.float32
    f16 = dt.float16
    bc = n_pairs * PAIR_N

    nc = bacc.Bacc("TRN2", target_bir_lowering=False, debug=False,
                   num_devices=N_CORES)

    xx_d = nc.dram_tensor("xx", [128, bc], f16, kind="ExternalInput")
    w1_d = nc.dram_tensor("w1", [128, N_S1, 128], f16, kind="ExternalInput")
    c1t_d = nc.dram_tensor("c1t", [128, KT], f32, kind="ExternalInput")
    w2_d = nc.dram_tensor("w2", [128, KT, 64], f16, kind="ExternalInput")
    w12_d = nc.dram_tensor("w12", [128, 128], f16, kind="ExternalInput")
    b2f_d = nc.dram_tensor("b2f", [128, 1], f32, kind="ExternalInput")
    w3_d = nc.dram_tensor("w3", [128, 64], f16, kind="ExternalInput")
    b3_d = nc.dram_tensor("b3", [64, 1], f32, kind="ExternalInput")
    w4_d = nc.dram_tensor("w4", [64, 17], f16, kind="ExternalInput")
    b4_d = nc.dram_tensor("b4", [17, 1], f32, kind="ExternalInput")
    w5_d = nc.dram_tensor("w5", [17, 1], f16, kind="ExternalInput")
    o_d = nc.dram_tensor("o", [1, bc], f32, kind="ExternalOutput")

    with tile.TileContext(nc) as tc, ExitStack() as ctx:
        wpool = ctx.enter_context(tc.tile_pool(name="wpool", bufs=1))
        xpool = ctx.enter_context(tc.tile_pool(name="xpool", bufs=3))
        ypool = ctx.enter_context(tc.tile_pool(name="ypool", bufs=10))
        spool = ctx.enter_context(tc.tile_pool(name="spool", bufs=2))
        ps1p = ctx.enter_context(tc.tile_pool(name="ps1p", bufs=3, space="PSUM"))
        ps2p = ctx.enter_context(tc.tile_pool(name="ps2p", bufs=1, space="PSUM"))

        # pair-0 input first so compute can start while the rest streams in.
        # w1 is split so the first slots' weights land quickly; w2 is split
        # so pair-0's first chain tiles don't wait for the whole load.
        xx_first = xpool.tile([128, PAIR_N], f16, tag="xx", name="xx_first")
        nc.sync.dma_start(xx_first[:, 0:CHUNK], xx_d[:, 0:CHUNK])
        nc.sync.dma_start(xx_first[:, CHUNK:PAIR_N], xx_d[:, CHUNK:PAIR_N])
        w1_t = wpool.tile([128, N_S1, 128], f16)
        nc.gpsimd.dma_start(w1_t[:, 0:1, :], w1_d[:, 0:1, :])
        nc.gpsimd.dma_start(w1_t[:, 1:3, :], w1_d[:, 1:3, :])
        c1t_t = wpool.tile([128, KT], f32)
        nc.gpsimd.dma_start(c1t_t[:, 0:6], c1t_d[:, 0:6])
        w12_t = wpool.tile([128, 128], f16)
        nc.gpsimd.dma_start(w12_t[:], w12_d[:])
        nc.gpsimd.dma_start(w1_t[:, 3:N_S1, :], w1_d[:, 3:N_S1, :])
        nc.gpsimd.dma_start(c1t_t[:, 6:KT], c1t_d[:, 6:KT])
        w2_t = wpool.tile([128, KT, 64], f16)
        nc.gpsimd.dma_start(w2_t[:, 0:8, :], w2_d[:, 0:8, :])
        nc.gpsimd.dma_start(w2_t[:, 8:KT, :], w2_d[:, 8:KT, :])
        b2f_t = wpool.tile([128, 1], f32)
        nc.gpsimd.dma_start(b2f_t[:], b2f_d[:])
        w3_t = wpool.tile([128, 64], f16)
        nc.gpsimd.dma_start(w3_t[:], w3_d[:])
        b3_t = wpool.tile([64, 1], f32)
        nc.gpsimd.dma_start(b3_t[:], b3_d[:])
        w4_t = wpool.tile([64, 17], f16)
        nc.gpsimd.dma_start(w4_t[:], w4_d[:])
        b4_t = wpool.tile([17, 1], f32)
        nc.gpsimd.dma_start(b4_t[:], b4_d[:])
        w5_t = wpool.tile([17, 1], f16)
        nc.gpsimd.dma_start(w5_t[:], w5_d[:])

        def make_tail_stages(p, h_t, final=False):
            """The per-pair serial tail (g1 -> g2 -> out) as stages that get
            interleaved into the NEXT pair's slot stream, so the ACT hops
            overlap matmuls instead of stalling the in-order PE queue.
            For the final pair (nothing left to interleave into) the
            activations are emitted per 512-chunk so chunk-1's activation
            overlaps chunk-0's next matmul instead of serializing."""
            st = {}
            acts = 2 if final else 1

            def ch(i):
                n = PAIR_N // acts
                return slice(i * n, (i + 1) * n)

            def s1():
                st["g1ps"] = ps1p.tile([64, PAIR_N], f32, tag="ps1",
                                       name=f"g1ps_{p}")
                for c in range(2):
                    sl = slice(c * CHUNK, (c + 1) * CHUNK)
                    nc.tensor.matmul(st["g1ps"][:, sl], w3_t[:],
                                     h_t[:, sl], start=True, stop=True)

            def s2():
                st["g1"] = spool.tile([64, PAIR_N], f16, tag="g1",
                                      name=f"g1_{p}")
                for i in range(acts):
                    nc.scalar.activation(st["g1"][:, ch(i)],
                                         st["g1ps"][:, ch(i)], AF.Lrelu,
                                         bias=b3_t[:, 0:1], alpha=LRELU_NEG)

            def s3():
                st["g2ps"] = ps1p.tile([17, PAIR_N], f32, tag="ps1",
                                       name=f"g2ps_{p}")
                for c in range(2):
                    sl = slice(c * CHUNK, (c + 1) * CHUNK)
                    nc.tensor.matmul(st["g2ps"][:, sl], w4_t[:],
                                     st["g1"][:, sl], start=True, stop=True)

            def s4():
                st["g2"] = spool.tile([17, PAIR_N], f16, tag="g2",
                                      name=f"g2_{p}")
                for i in range(acts):
                    nc.scalar.activation(st["g2"][:, ch(i)],
                                         st["g2ps"][:, ch(i)], AF.Lrelu,
                                         bias=b4_t[:, 0:1], alpha=LRELU_NEG)

            def s5():
                st["ops"] = ps1p.tile([1, PAIR_N], f32, tag="ps1",
                                      name=f"ops_{p}")
                for c in range(2):
                    sl = slice(c * CHUNK, (c + 1) * CHUNK)
                    nc.tensor.matmul(st["ops"][:, sl], w5_t[:],
                                     st["g2"][:, sl], start=True, stop=True)

            def s6():
                # single-partition copy is ~1.1us either way; Scalar has the
                # headroom (Vector owns the buffer-critical A evacuations)
                o_t = spool.tile([1, PAIR_N], f32, tag="o", name=f"o_{p}")
                for i in range(acts):
                    nc.scalar.activation(o_t[:, ch(i)], st["ops"][:, ch(i)],
                                         AF.Copy)
                nc.sync.dma_start(o_d[:, p * PAIR_N:(p + 1) * PAIR_N], o_t[:])

            return [s1, s2, s3, s4, s5, s6]

        tail_stages = []

        for p in range(n_pairs):
            if p == 0:
                xx_t = xx_first
            else:
                xx_t = xpool.tile([128, PAIR_N], f16, tag="xx", name=f"xx_{p}")
                nc.sync.dma_start(xx_t[:],
                                  xx_d[:, p * PAIR_N:(p + 1) * PAIR_N])

            # step-2 accumulator: single [128, 1024] PSUM tile; each 512
            # chunk holds two concurrent M=64 accumulation chains (group A
            # in partitions 0:64, group B in 64:128).
            ps2 = ps2p.tile([128, PAIR_N], f32, tag="ps2", name=f"ps2_{p}")

            def _emit_chains(tiles_done):
                if len(tiles_done) == 2 and GRP[tiles_done[0]] != GRP[tiles_done[1]]:
                    # A/B pair: order so adjacent matmuls sit in opposite
                    # column groups and dual-stream
                    t0, t1 = tiles_done
                    order = [(t0, 0), (t1, 0), (t1, 1), (t0, 1)]
                else:
                    order = [(t, c) for t in tiles_done for c in range(2)]
                for t, c in order:
                    g = GRP[t]
                    sl = slice(c * CHUNK, (c + 1) * CHUNK)
                    nc.tensor.matmul(
                        ps2[64 * g:64 * (g + 1), sl], w2_t[:, t, :],
                        ytiles[t][:, sl], start=False,
                        stop=(t == GROUP_LAST_T[g]),
                        tile_position=(0, 64 * g), skip_group_check=True)

            ytiles = [None] * KT
            pending = []
            for s in range(N_S1):
                # tail stages start at slot 2: the first stage's matmuls
                # depend on the h activation queued at the pair boundary,
                # and popping it earlier parks the in-order PE queue.
                if s >= 2 and tail_stages:
                    tail_stages.pop(0)()
                tA, tB = 2 * s, 2 * s + 1
                psA = ps1p.tile([128, PAIR_N], f32, tag="ps1",
                                name=f"psA_{p}_{s}")
                if tB < KT:
                    psB = ps1p.tile([128, PAIR_N], f32, tag="ps1",
                                    name=f"psB_{p}_{s}")
                # alternate row halves so consecutive matmuls dual-stream
                for c in range(2):
                    sl = slice(c * CHUNK, (c + 1) * CHUNK)
                    nc.tensor.matmul(
                        psA[:, sl], w1_t[0:64, s, :], xx_t[0:64, sl],
                        start=True, stop=True, tile_position=(0, 0))
                    if tB < KT:
                        nc.tensor.matmul(
                            psB[:, sl], w1_t[64:128, s, :], xx_t[64:128, sl],
                            start=True, stop=True, tile_position=(64, 0))
                done = []
                for t, ps in ((tA, psA),) + (((tB, psB),) if tB < KT else ()):
                    y_t = ypool.tile([128, PAIR_N], f16, tag="y",
                                     name=f"y_{p}_{t}")
                    if DVE_TILE[t]:
                        nc.vector.tensor_scalar(
                            y_t[:], ps[:], c1t_t[:, t:t + 1], 0.0,
                            ALU.add, ALU.max)
                    else:
                        nc.scalar.activation(
                            y_t[:], ps[:], AF.Relu, bias=c1t_t[:, t:t + 1])
                    ytiles[t] = y_t
                    done.append(t)
                pending.append(done)
                if s == CHAIN_LAG:
                    # full-width K=128 chain opener (row-duplicated W12):
                    # needs only xx, and its M=128 write sets has_written
                    # across both column groups' partition ranges.  Emitted
                    # at the first chain slot (still before the w2 chains
                    # below) so it never waits on the previous pair's h
                    # activation freeing the single-buffered ps2.
                    for c in range(2):
                        sl = slice(c * CHUNK, (c + 1) * CHUNK)
                        nc.tensor.matmul(
                            ps2[:, sl], w12_t[:], xx_t[:, sl],
                            start=True, stop=False, skip_group_check=True)
                if len(pending) > CHAIN_LAG:
                    _emit_chains(pending.pop(0))
            for done in pending:
                _emit_chains(done)

            # any remaining previous-pair tail stages
            for st in tail_stages:
                st()
            # h evacuation immediately at pair end: frees ps2 for the next
            # pair's chain opener (ps2 is single-buffered)
            h_t = spool.tile([128, PAIR_N], f16, tag="h", name=f"h_{p}")
            nc.scalar.activation(h_t[:], ps2[:], AF.Lrelu,
                                 bias=b2f_t[:, 0:1], alpha=LRELU_NEG)
            tail_stages = make_tail_stages(p, h_t, final=(p == n_pairs - 1))

        for st in tail_stages:
            st()

    nc.compile()
    return nc


# ---------------------------------------------------------------- execution
_NC_CACHE = {}
LAST_RESULT = None


def _prep_inputs(inputs):
    board = np.ascontiguousarray(np.asarray(inputs["board"], np.float32))
    x = board.reshape(B_TOTAL, 64)
    dev = _fold_params(inputs)
    in_maps = []
    for c in range(N_CORES):
        xc = np.ascontiguousarray(x[c * BC:(c + 1) * BC].T)      # [64, BC]
        m = dict(dev)
        m["xx"] = np.ascontiguousarray(
            np.vstack([xc, xc]).astype(np.float16))              # [128, BC]
        in_maps.append(m)
    return in_maps


def kernel(**inputs):
    global LAST_RESULT
    from concourse.bass_utils import run_bass_kernel_spmd

    if "nc" not in _NC_CACHE:
        _NC_CACHE["nc"] = _build_nc()
    nc = _NC_CACHE["nc"]

    in_maps = _prep_inputs(inputs)
    res = run_bass_kernel_spmd(nc, in_maps, core_ids=list(range(N_CORES)))
    LAST_RESULT = res
    out = np.concatenate([r["o"].reshape(-1) for r in res.results])
    return out.reshape(B_TOTAL, 1).astype(np.float32)
